# revision 19
# baseline (speedup 1.0000x reference)
"""Trainium2 Bass kernel for NeuronLlama4VisionMLP (fused residual-add +
RMSNorm + up-proj + GELU + down-proj).

Distribution: data-parallel over the 16384 tokens -> 2048 tokens per core,
full weights replicated per core, no collectives.

Host side (cheap elementwise / repack prep):
  - h = x + residual  (this is also the module's second output)
  - per-token rsqrt(mean(h^2)+eps) scale and ln_w are folded into the
    device inputs: normed = h * s, W_up' = ln_w[:,None] * W_up
  - normed is shipped transposed + chunk-repacked fp16 so each chunk is
    one fully HBM-contiguous DMA; the device returns out^T per m-tile and
    b_down is added on host.

Device side per core (T=2048 tokens, H=1408 -> KH=11 tiles, I=5632 ->
KI=44 tiles), processed as 4 passes over 512-token chunks; everything in
fp16 so every matmul runs at the 1 col/cycle @2.4GHz peak (f32r weights
pay ~+11ns/MM in LDWEIGHTS):
    up:   psum[i, c] = sum_k wup[k, i].T @ nt[k, c]      (11-MM chains)
    gelu: act[i, c] = Gelu(psum + b_up[i])               (scalar engine)
    down: psum[m, c] = sum_i wdn[i, m].T @ act[i, c]     (44-MM chains)
    out^T[m, c] -> HBM (f32)

Weights are re-streamed per chunk pass (4x wup, 4x wdn ~ 127MB/core
total) -- the ~360GB/s/core HBM fabric absorbs it, so never the
bottleneck. In exchange act SBUF drops to 5.9MB and chain 0 starts at
~20us (vs 25us baseline) with the HAM clock pre-warmed.

Scheduling learned the hard way (see per-queue notes in build_bass):
the three DMA queues (sync/scalar/gpsimd) share ~360GB/s; weight tiles
move as full-tile dense descriptors; the prologue is a hand-ordered
wavefront with a NWARM junk-matmul burst bridging until data arrives so
the PE clock gate (1.2->2.4GHz) never re-cools; wdn/nt prefetches are
gated to late gelus so they can't dilute the pass-0 wup catch-up
stream; outs ride sync; psum->sbuf copies ride the idle vector engine;
scalar runs the gelus.
"""
import sys

sys.path.insert(0, "/opt/trn_rl_repo")

import numpy as np
import ml_dtypes
import concourse.bass as bass
from concourse import bacc
import concourse.mybir as mybir
from concourse.tile import TileContext
from concourse.bass_utils import run_bass_kernel_spmd

# Problem shape (hardcoded per contract)
B, S, H, I = 16, 1024, 1408, 5632
EPS = 1e-6
NCORES = 8
P = 128
T_CORE = (B * S) // NCORES       # 2048 tokens per core
KH = H // P                      # 11 k-tiles of H
KI = I // P                      # 44 k-tiles of I
IC = 4                           # i-chunks in down weight repack
ISUB = KI // IC                  # 11 i-subtiles per chunk
CH = 512                         # token chunk width (= max fp32-psum MM N)
NCH = T_CORE // CH               # 4 chunk passes per core
NTSPLIT = 6                      # nt chunk DMA k-split
NWARM = 15                       # HAM warmup matmuls
NPRE = 8                         # wup pair-tiles prefetched in the prologue

F16 = mybir.dt.float16
F32 = mybir.dt.float32




def build_bass():
    nc = bacc.Bacc(None, target_bir_lowering=False)

    # all weight/act DMAs are HBM-contiguous with multi-KB per-partition runs
    nt = nc.declare_dram_parameter("nt", [NCH, P, KH, CH], F16, isOutput=False)
    wup = nc.declare_dram_parameter("wup", [KI // 2, P, 2, KH, P], F16, isOutput=False)
    wdn = nc.declare_dram_parameter("wdn", [KH, P, IC, ISUB, P], F16, isOutput=False)
    # host-transposed [P, KI] so the DMA is one contiguous run per
    # partition; a "(i p) -> p i" rearrange of the flat vector emits 5632
    # single-element descriptors that clog the issuing queue for ~20us
    bup = nc.declare_dram_parameter("bup", [P, KI], F32, isOutput=False)
    ot = nc.declare_dram_parameter("ot", [KH, P, T_CORE], F32, isOutput=True)

    with TileContext(nc) as tc:
        with (
            tc.tile_pool(name="const", bufs=1) as constp,
            tc.tile_pool(name="ntp", bufs=2) as ntp,
            tc.tile_pool(name="wupp", bufs=NPRE) as wupp,
            tc.tile_pool(name="wdnp", bufs=4) as wdnp,
            tc.tile_pool(name="actp", bufs=KI + 2) as actp,
            tc.tile_pool(name="outp", bufs=4) as outp,
            tc.tile_pool(name="psu", bufs=4, space="PSUM") as psu,
            tc.tile_pool(name="psd", bufs=4, space="PSUM") as psd,
        ):
            # Measured per-queue DMA throughput: gpsimd (SWDGE) ~200GB/s;
            # sync/scalar (HWDGE) only ~70-90GB/s each. Transfers issued on
            # one queue share its bandwidth concurrently, and throughput
            # also degrades below ~2KB per-partition runs. So: steady-state
            # weight tiles move as full-tile single descriptors (5.6-11.3KB
            # runs) weighted 2:1 toward gpsimd, and the prologue hand-
            # schedules the first ~3MB by need-time with the critical bytes
            # leading the gpsimd queue. Non-critical prefetches are gated
            # behind the first gelus (the in-order scalar queue can't reach
            # their dma_starts before the preceding ACTIVATE retires).
            nt_tiles = {}

            # The Tile scheduler orders instructions by dependency, not by
            # python issue order: a "gelu-gated" dma_start with free pool
            # slots gets hoisted to t0 and its bytes then compete with the
            # critical prologue stream (seen in traces: wdn0/nt1 dispatch
            # at ~10us). gate_on() forces a real dependency: a 1-element
            # copy from the gate tile into the prefetch tile makes the
            # full-tile DMA WAW-dependent on the gate tile's producer.
            def fetch_nt(c, gate=None):
                t = ntp.tile([P, KH, CH], F16, tag="nt", name=f"nt{c}")
                if gate is not None:
                    nc.vector.tensor_copy(out=t[0:1, 0:1, 0:1], in_=gate[0:1, 0:1])
                nc.scalar.dma_start(out=t[:], in_=nt[c])
                nt_tiles[c] = t

            def issue_wup(ip, eng):
                t = wupp.tile([P, 2, KH, P], F16, tag="wup", name=f"wup{ip}")
                eng.dma_start(out=t[:], in_=wup[ip])
                return t

            def issue_wdn(m, eng, gate=None):
                t = wdnp.tile([P, IC, ISUB, P], F16, tag="wdn", name=f"wdn{m}")
                if gate is not None:
                    nc.vector.tensor_copy(
                        out=t[0:1, 0:1, 0:1, 0:1], in_=gate[0:1, 0:1]
                    )
                eng.dma_start(out=t[:], in_=wdn[m])
                return t

            # ---- prologue: critical-first, staggered so chain 0 streams
            # data-paced from ~12us. Measured early per-queue shares while
            # all three compete: gpsimd ~165, scalar ~105, sync ~55 GB/s
            # (sync is the runt -- it gets only bup + the last k-slice).
            # wup[0] is split per-half so chain 0 (half 0) isn't gated on
            # the full pair; nt0 lands in five staggered slices roughly in
            # k-order so chain 0's k-MMs trickle in behind the junk warmup.
            # The remaining prologue pairs wup[1..NPRE-1] follow in
            # need-order (one pair per 4.75us); in-loop pairs are
            # slot-gated by the NPRE-deep pool (~35us of lead).
            wup_pre = {}
            nt0 = ntp.tile([P, KH, CH], F16, tag="nt", name="nt0")
            nt_tiles[0] = nt0
            wpre = []
            for ip in range(NPRE):
                wpre.append(wupp.tile([P, 2, KH, P], F16, tag="wup", name=f"wup{ip}"))
                wup_pre[ip] = wpre[ip]
            bup_sb = constp.tile([P, KI], F32)
            # critical bytes (nt0 + the wup[0] pair + bup) lead each queue;
            # pieces kept >=0.35MB (smaller slices pay ~0.7us per-DMA
            # completion overhead per queue and tank queue throughput)
            nc.gpsimd.dma_start(out=nt0[:, 0:3], in_=nt[0][:, 0:3])
            nc.scalar.dma_start(out=wpre[0][:, 0], in_=wup[0][:, 0])
            nc.sync.dma_start(out=bup_sb[:], in_=bup[:, 0:KI])
            nc.sync.dma_start(out=nt0[:, 9:KH], in_=nt[0][:, 9:KH])
            nc.gpsimd.dma_start(out=nt0[:, 3:6], in_=nt[0][:, 3:6])
            nc.scalar.dma_start(out=wpre[0][:, 1], in_=wup[0][:, 1])
            nc.scalar.dma_start(out=nt0[:, 6:9], in_=nt[0][:, 6:9])
            # rest of the prologue wup pairs, need-ordered across queues.
            # Each is gated on a byte of a critical piece (1-elem copy ->
            # WAW dep) so the SDMA engines don't round-robin any of these
            # 0.72MB pairs ahead of the critical tail; they dispatch as the
            # criticals complete, still ~5-10us before their chains.
            def gate_pair(ip, gate_ap):
                nc.vector.tensor_copy(out=wpre[ip][0:1, 0:1, 0:1, 0:1], in_=gate_ap)

            g_early = nt0[0:1, 0:1, 0:1]     # lands ~13us
            g_mid = nt0[0:1, 3:4, 0:1]       # lands ~15us
            g_late = nt0[0:1, 6:7, 0:1]      # lands ~17us
            gate_pair(1, g_early)
            nc.gpsimd.dma_start(out=wpre[1][:], in_=wup[1])
            gate_pair(2, g_early)
            nc.scalar.dma_start(out=wpre[2][:], in_=wup[2])
            gate_pair(3, g_mid)
            nc.gpsimd.dma_start(out=wpre[3][:], in_=wup[3])
            gate_pair(4, g_mid)
            nc.sync.dma_start(out=wpre[4][:], in_=wup[4])
            gate_pair(5, g_late)
            nc.gpsimd.dma_start(out=wpre[5][:], in_=wup[5])
            gate_pair(6, g_late)
            nc.scalar.dma_start(out=wpre[6][:], in_=wup[6])
            gate_pair(7, g_late)
            nc.sync.dma_start(out=wpre[7][:], in_=wup[7])

            # PE warmup: ~10 junk matmuls flip the HAM clock gate
            # (1.2->2.4GHz) while the prologue DMAs are still in flight,
            # so the first real chains run at full clock.
            wa = constp.tile([P, CH], F16)
            wb = constp.tile([P, P], F16)
            nc.vector.memset(wa[:], 0.0)
            nc.vector.memset(wb[:], 0.0)
            for _ in range(NWARM):
                pw = psu.tile([P, CH], F32, tag="psu", name="pw")
                nc.tensor.matmul(pw[:], wb[:], wa[:], start=True, stop=True)

            for c in range(NCH):
                ntc = nt_tiles.pop(c)

                # ---- up projection + gelu over chunk c ----
                acts = []
                wdn_pre = []
                for ip in range(KI // 2):
                    if ip in wup_pre:
                        wupb = wup_pre.pop(ip)
                    else:
                        wupb = issue_wup(ip, nc.sync if ip % 4 == 0 else nc.gpsimd)
                    for half in range(2):
                        i = 2 * ip + half
                        ps = psu.tile([P, CH], F32, tag="psu")
                        for k in range(KH):
                            nc.tensor.matmul(
                                ps[:],
                                wupb[:, half, k],
                                ntc[:, k],
                                start=(k == 0),
                                stop=(k == KH - 1),
                            )
                        a = actp.tile([P, CH], F16, tag="act", name=f"act{i}")
                        nc.scalar.activation(
                            a[:],
                            ps[:],
                            mybir.ActivationFunctionType.Gelu,
                            bias=bup_sb[:, i : i + 1],
                            scale=1.0,
                        )
                        acts.append(a)
                        # gelu-gated prefetches: wdn/nt are needed only at
                        # the down phase (~100us later); gating them at late
                        # gelus keeps the early fabric bandwidth for the wup
                        # stream. The wup stream itself is slot-gated via
                        # the NPRE-deep pool (~35us of lead), so it needs no
                        # gelu gating.
                        # On chunk 0 the wdnp/ntp pool slots are empty, so
                        # every un-gated wdn/nt prefetch would hoist to t0;
                        # chunk 0 therefore gates wdn0..3 explicitly. Later
                        # chunks are naturally gated by slot reuse.
                        sched = {12: "wdn0", 18: "wdn1", 24: "wdn2", 30: "nt"}
                        if c == 0:
                            sched[36] = "wdn3"
                        ev = sched.get(i)
                        if ev == "nt":
                            if c + 1 < NCH:
                                fetch_nt(c + 1, gate=a)
                        elif ev == "wdn0":
                            wdn_pre.append(issue_wdn(0, nc.scalar, gate=a))
                        elif ev == "wdn1":
                            wdn_pre.append(issue_wdn(1, nc.scalar, gate=a))
                        elif ev == "wdn2":
                            wdn_pre.append(issue_wdn(2, nc.gpsimd, gate=a))
                        elif ev == "wdn3":
                            wdn_pre.append(issue_wdn(3, nc.gpsimd, gate=a))

                # ---- down projection over chunk c ----
                for m in range(KH):
                    if m < len(wdn_pre):
                        wdnb = wdn_pre[m]
                    else:
                        wdnb = issue_wdn(m, nc.scalar if m % 3 == 0 else nc.gpsimd)
                    ps2 = psd.tile([P, CH], F32, tag="psd")
                    for i in range(KI):
                        nc.tensor.matmul(
                            ps2[:],
                            wdnb[:, i // ISUB, i % ISUB],
                            acts[i][:],
                            start=(i == 0),
                            stop=(i == KI - 1),
                        )
                    osb = outp.tile([P, CH], F32, tag="osb")
                    tok = slice(c * CH, (c + 1) * CH)
                    out_eng = nc.sync
                    if c == NCH - 1 and m == KH - 1:
                        # split the final output so copy/DMA pipeline
                        HC = CH // 2
                        for q in range(2):
                            lo, hi = q * HC, (q + 1) * HC
                            nc.vector.tensor_copy(out=osb[:, lo:hi], in_=ps2[:, lo:hi])
                            out_eng.dma_start(
                                out=ot[m][:, c * CH + lo : c * CH + hi],
                                in_=osb[:, lo:hi],
                            )
                    else:
                        nc.vector.tensor_copy(out=osb[:], in_=ps2[:])
                        out_eng.dma_start(out=ot[m][:, tok], in_=osb[:])
    nc.compile()
    return nc


_CACHED = {}


def _get_nc():
    if "nc" not in _CACHED:
        _CACHED["nc"] = build_bass()
    return _CACHED["nc"]


def _prep_host(x, residual, ln_w, W_up, b_up, W_down):
    """Host-side prep: h, chunk-repacked fp16 normed^T per core, weights."""
    h = x + residual                                   # [B,S,H] f32
    hf = h.reshape(-1, H)                              # [16384, H]
    var = np.mean(np.square(hf), axis=-1)              # f32
    s = 1.0 / np.sqrt(var + EPS)                       # f32
    normed = (hf * s[:, None]).astype(np.float16)      # ln_w folded into W

    Wup_p = (W_up * ln_w[:, None]).astype(np.float16)  # [H, I]
    # wup[ip, p, b, k, il] = Wup_p[k*128+p, (2*ip+b)*128+il]
    WUP = np.ascontiguousarray(
        Wup_p.reshape(KH, P, KI // 2, 2, P).transpose(2, 1, 3, 0, 4)
    )                                                  # [KI/2,P,2,KH,P] f16
    # wdn[m, p, ic, isub, cc] = W_down[(ic*ISUB+isub)*128+p, m*128+cc]
    WDN = np.ascontiguousarray(
        W_down.astype(np.float16).reshape(IC, ISUB, P, KH, P).transpose(3, 2, 0, 1, 4)
    )                                                  # [KH,P,IC,ISUB,P] f16

    # bup[p, i] = b_up[i*128+p] -> contiguous [P, KI] DMA
    BUP = np.ascontiguousarray(b_up.astype(np.float32).reshape(KI, P).T)

    in_maps = []
    for c in range(NCORES):
        blk = normed[c * T_CORE : (c + 1) * T_CORE]    # [T_CORE, H] f16
        # nt[ch, p, k, j] = normed^T[k*128+p, ch*512+j]
        ntc = np.ascontiguousarray(
            blk.T.reshape(KH, P, NCH, CH).transpose(2, 1, 0, 3)
        )                                              # [NCH,P,KH,CH] f16
        in_maps.append({"nt": ntc, "wup": WUP, "wdn": WDN, "bup": BUP})
    return h, in_maps


_RESET_DONE = {}


def _maybe_reset_device():
    """Best-effort terminal NRT reset so a previously wedged device can't
    hang the run. No-op when the axon .so or symbol is unavailable."""
    if _RESET_DONE:
        return
    _RESET_DONE["done"] = True
    try:
        import ctypes
        import jax

        jax.devices()
        lib = ctypes.CDLL("/opt/axon/libaxon_pjrt.so")
        if hasattr(lib, "axon_reset"):
            lib.axon_reset.restype = ctypes.c_int64
            lib.axon_reset()
    except Exception:
        pass


def _run(in_maps, **kw):
    _maybe_reset_device()
    nc = _get_nc()
    return run_bass_kernel_spmd(nc, in_maps, core_ids=list(range(NCORES)), **kw)


def _assemble(results, b_down):
    # ot[m, p, t] -> out[t, m*128+p]
    outs = [r["ot"].transpose(2, 0, 1).reshape(T_CORE, H) for r in results]
    out = np.concatenate(outs, axis=0).reshape(B, S, H)
    out = out + b_down.astype(np.float32)
    return out


def kernel(x, residual, ln_w, W_up, b_up, W_down, b_down):
    x = np.asarray(x, dtype=np.float32)
    residual = np.asarray(residual, dtype=np.float32)
    ln_w = np.asarray(ln_w, dtype=np.float32)
    W_up = np.asarray(W_up, dtype=np.float32)
    b_up = np.asarray(b_up, dtype=np.float32)
    W_down = np.asarray(W_down, dtype=np.float32)
    b_down = np.asarray(b_down, dtype=np.float32)

    h, in_maps = _prep_host(x, residual, ln_w, W_up, b_up, W_down)
    res = _run(in_maps)
    out = _assemble(res.results, b_down)
    return out, h


def kernel_traced(x, residual, ln_w, W_up, b_up, W_down, b_down, **kw):
    """Like kernel() but with NTFF tracing; returns ((out, h), results)."""
    h, in_maps = _prep_host(
        np.asarray(x, np.float32),
        np.asarray(residual, np.float32),
        np.asarray(ln_w, np.float32),
        np.asarray(W_up, np.float32),
        np.asarray(b_up, np.float32),
        np.asarray(W_down, np.float32),
    )
    res = _run(in_maps, trace=True, **kw)
    out = _assemble(res.results, np.asarray(b_down, np.float32))
    return (out, h), res



# revision 20
# speedup vs baseline: 1.0902x; 1.0902x over previous
"""Trainium2 Bass kernel for NeuronLlama4VisionMLP (fused residual-add +
RMSNorm + up-proj + GELU + down-proj).

Distribution: data-parallel over the 16384 tokens -> 2048 tokens per core,
full weights replicated per core, no collectives.

Host side (cheap elementwise / repack prep):
  - h = x + residual  (this is also the module's second output)
  - per-token rsqrt(mean(h^2)+eps) scale and ln_w are folded into the
    device inputs: normed = h * s, W_up' = ln_w[:,None] * W_up
  - normed is shipped transposed + chunk-repacked fp16 so each chunk is
    one fully HBM-contiguous DMA; the device returns out^T per m-tile and
    b_down is added on host.

Device side per core (T=2048 tokens, H=1408 -> KH=11 tiles, I=5632 ->
KI=44 tiles), processed as 4 passes over 512-token chunks; everything in
fp16 so every matmul runs at the 1 col/cycle @2.4GHz peak (f32r weights
pay ~+11ns/MM in LDWEIGHTS):
    up:   psum[i, c] = sum_k wup[k, i].T @ nt[k, c]      (11-MM chains)
    gelu: act[i, c] = Gelu(psum + b_up[i])               (scalar engine)
    down: psum[m, c] = sum_i wdn[i, m].T @ act[i, c]     (44-MM chains)
    out^T[m, c] -> HBM (f32)

Weights are re-streamed per chunk pass (4x wup, 4x wdn ~ 127MB/core
total) -- the ~360GB/s/core HBM fabric absorbs it, so never the
bottleneck. In exchange act SBUF drops to 5.9MB.

Scheduling learned the hard way (see per-queue notes in build_bass):
the three DMA queues (sync/scalar/gpsimd) share ~360GB/s round-robin
at packet granularity, so queue ORDER alone cannot prioritize -- and
the Tile scheduler hoists any dma_start with free pool slots to t0
regardless of python issue order. Every non-critical prefetch is
therefore gated by a real dependency (1-element vector copy into the
prefetch tile -> WAW dep on its DMA): prologue wup pairs gate on bytes
of the critical nt0/w0 pieces, wdn/nt chunk prefetches gate on mid-
chunk gelu outputs. Chain 0 then streams data-paced from ~12us while
NWARM junk matmuls bridge the HAM clock gate (1.2->2.4GHz) until the
real MM stream is dense (~15us); total mm-stream gaps measure ~2us
over the 836us span. Critical prologue pieces are kept >=0.35MB (per-
DMA completion costs ~0.7us/queue) and balanced to the measured queue
shares (gpsimd ~165, scalar ~105, sync ~55-90 GB/s). Outs ride sync;
psum->sbuf copies ride the idle vector engine; scalar runs the gelus.
Note: back-to-back runs trip the P0 power throttle (PE 2.4->2.0GHz,
+20% wall); benchmark on a cool device.
"""
import sys

sys.path.insert(0, "/opt/trn_rl_repo")

import numpy as np
import ml_dtypes
import concourse.bass as bass
from concourse import bacc
import concourse.mybir as mybir
from concourse.tile import TileContext
from concourse.bass_utils import run_bass_kernel_spmd

# Problem shape (hardcoded per contract)
B, S, H, I = 16, 1024, 1408, 5632
EPS = 1e-6
NCORES = 8
P = 128
T_CORE = (B * S) // NCORES       # 2048 tokens per core
KH = H // P                      # 11 k-tiles of H
KI = I // P                      # 44 k-tiles of I
IC = 4                           # i-chunks in down weight repack
ISUB = KI // IC                  # 11 i-subtiles per chunk
CH = 512                         # token chunk width (= max fp32-psum MM N)
NCH = T_CORE // CH               # 4 chunk passes per core
NTSPLIT = 6                      # nt chunk DMA k-split
NWARM = 15                       # HAM warmup matmuls
NPRE = 8                         # wup pair-tiles prefetched in the prologue

F16 = mybir.dt.float16
F32 = mybir.dt.float32




def build_bass():
    nc = bacc.Bacc(None, target_bir_lowering=False)

    # all weight/act DMAs are HBM-contiguous with multi-KB per-partition runs
    nt = nc.declare_dram_parameter("nt", [NCH, P, KH, CH], F16, isOutput=False)
    wup = nc.declare_dram_parameter("wup", [KI // 2, P, 2, KH, P], F16, isOutput=False)
    wdn = nc.declare_dram_parameter("wdn", [KH, P, IC, ISUB, P], F16, isOutput=False)
    # host-transposed [P, KI] so the DMA is one contiguous run per
    # partition; a "(i p) -> p i" rearrange of the flat vector emits 5632
    # single-element descriptors that clog the issuing queue for ~20us
    bup = nc.declare_dram_parameter("bup", [P, KI], F32, isOutput=False)
    ot = nc.declare_dram_parameter("ot", [KH, P, T_CORE], F32, isOutput=True)

    with TileContext(nc) as tc:
        with (
            tc.tile_pool(name="const", bufs=1) as constp,
            tc.tile_pool(name="ntp", bufs=2) as ntp,
            tc.tile_pool(name="wupp", bufs=NPRE) as wupp,
            tc.tile_pool(name="wdnp", bufs=4) as wdnp,
            tc.tile_pool(name="actp", bufs=KI + 2) as actp,
            tc.tile_pool(name="outp", bufs=4) as outp,
            tc.tile_pool(name="psu", bufs=4, space="PSUM") as psu,
            tc.tile_pool(name="psd", bufs=4, space="PSUM") as psd,
        ):
            # Measured per-queue DMA throughput: gpsimd (SWDGE) ~200GB/s;
            # sync/scalar (HWDGE) only ~70-90GB/s each. Transfers issued on
            # one queue share its bandwidth concurrently, and throughput
            # also degrades below ~2KB per-partition runs. So: steady-state
            # weight tiles move as full-tile single descriptors (5.6-11.3KB
            # runs) weighted 2:1 toward gpsimd, and the prologue hand-
            # schedules the first ~3MB by need-time with the critical bytes
            # leading the gpsimd queue. Non-critical prefetches are gated
            # behind the first gelus (the in-order scalar queue can't reach
            # their dma_starts before the preceding ACTIVATE retires).
            nt_tiles = {}

            # The Tile scheduler orders instructions by dependency, not by
            # python issue order: a "gelu-gated" dma_start with free pool
            # slots gets hoisted to t0 and its bytes then compete with the
            # critical prologue stream (seen in traces: wdn0/nt1 dispatch
            # at ~10us). gate_on() forces a real dependency: a 1-element
            # copy from the gate tile into the prefetch tile makes the
            # full-tile DMA WAW-dependent on the gate tile's producer.
            def fetch_nt(c, gate=None):
                t = ntp.tile([P, KH, CH], F16, tag="nt", name=f"nt{c}")
                if gate is not None:
                    nc.vector.tensor_copy(out=t[0:1, 0:1, 0:1], in_=gate[0:1, 0:1])
                nc.scalar.dma_start(out=t[:], in_=nt[c])
                nt_tiles[c] = t

            def issue_wup(ip, eng):
                t = wupp.tile([P, 2, KH, P], F16, tag="wup", name=f"wup{ip}")
                eng.dma_start(out=t[:], in_=wup[ip])
                return t

            def issue_wdn(m, eng, gate=None):
                t = wdnp.tile([P, IC, ISUB, P], F16, tag="wdn", name=f"wdn{m}")
                if gate is not None:
                    nc.vector.tensor_copy(
                        out=t[0:1, 0:1, 0:1, 0:1], in_=gate[0:1, 0:1]
                    )
                eng.dma_start(out=t[:], in_=wdn[m])
                return t

            # ---- prologue: critical-first, staggered so chain 0 streams
            # data-paced from ~12us. Measured early per-queue shares while
            # all three compete: gpsimd ~165, scalar ~105, sync ~55 GB/s
            # (sync is the runt -- it gets only bup + the last k-slice).
            # wup[0] is split per-half so chain 0 (half 0) isn't gated on
            # the full pair; nt0 lands in five staggered slices roughly in
            # k-order so chain 0's k-MMs trickle in behind the junk warmup.
            # The remaining prologue pairs wup[1..NPRE-1] follow in
            # need-order (one pair per 4.75us); in-loop pairs are
            # slot-gated by the NPRE-deep pool (~35us of lead).
            wup_pre = {}
            nt0 = ntp.tile([P, KH, CH], F16, tag="nt", name="nt0")
            nt_tiles[0] = nt0
            wpre = []
            for ip in range(NPRE):
                wpre.append(wupp.tile([P, 2, KH, P], F16, tag="wup", name=f"wup{ip}"))
                wup_pre[ip] = wpre[ip]
            bup_sb = constp.tile([P, KI], F32)
            # critical bytes (nt0 + the wup[0] pair + bup) lead each queue;
            # pieces kept >=0.35MB (smaller slices pay ~0.7us per-DMA
            # completion overhead per queue and tank queue throughput)
            nc.gpsimd.dma_start(out=nt0[:, 0:3], in_=nt[0][:, 0:3])
            nc.scalar.dma_start(out=wpre[0][:, 0], in_=wup[0][:, 0])
            nc.sync.dma_start(out=bup_sb[:], in_=bup[:, 0:KI])
            nc.sync.dma_start(out=nt0[:, 9:KH], in_=nt[0][:, 9:KH])
            nc.gpsimd.dma_start(out=nt0[:, 3:6], in_=nt[0][:, 3:6])
            nc.scalar.dma_start(out=wpre[0][:, 1], in_=wup[0][:, 1])
            nc.scalar.dma_start(out=nt0[:, 6:9], in_=nt[0][:, 6:9])
            # rest of the prologue wup pairs, need-ordered across queues.
            # Each is gated on a byte of a critical piece (1-elem copy ->
            # WAW dep) so the SDMA engines don't round-robin any of these
            # 0.72MB pairs ahead of the critical tail; they dispatch as the
            # criticals complete, still ~5-10us before their chains.
            def gate_pair(ip, gate_ap):
                nc.vector.tensor_copy(out=wpre[ip][0:1, 0:1, 0:1, 0:1], in_=gate_ap)

            g_early = nt0[0:1, 0:1, 0:1]     # lands ~13us
            g_mid = nt0[0:1, 3:4, 0:1]       # lands ~15us
            g_late = nt0[0:1, 6:7, 0:1]      # lands ~17us
            gate_pair(1, g_early)
            nc.gpsimd.dma_start(out=wpre[1][:], in_=wup[1])
            gate_pair(2, g_early)
            nc.scalar.dma_start(out=wpre[2][:], in_=wup[2])
            gate_pair(3, g_mid)
            nc.gpsimd.dma_start(out=wpre[3][:], in_=wup[3])
            gate_pair(4, g_mid)
            nc.sync.dma_start(out=wpre[4][:], in_=wup[4])
            gate_pair(5, g_late)
            nc.gpsimd.dma_start(out=wpre[5][:], in_=wup[5])
            gate_pair(6, g_late)
            nc.scalar.dma_start(out=wpre[6][:], in_=wup[6])
            gate_pair(7, g_late)
            nc.sync.dma_start(out=wpre[7][:], in_=wup[7])

            # PE warmup: ~10 junk matmuls flip the HAM clock gate
            # (1.2->2.4GHz) while the prologue DMAs are still in flight,
            # so the first real chains run at full clock.
            wa = constp.tile([P, CH], F16)
            wb = constp.tile([P, P], F16)
            nc.vector.memset(wa[:], 0.0)
            nc.vector.memset(wb[:], 0.0)
            for _ in range(NWARM):
                pw = psu.tile([P, CH], F32, tag="psu", name="pw")
                nc.tensor.matmul(pw[:], wb[:], wa[:], start=True, stop=True)

            for c in range(NCH):
                ntc = nt_tiles.pop(c)

                # ---- up projection + gelu over chunk c ----
                acts = []
                wdn_pre = []
                for ip in range(KI // 2):
                    if ip in wup_pre:
                        wupb = wup_pre.pop(ip)
                    else:
                        wupb = issue_wup(ip, nc.sync if ip % 4 == 0 else nc.gpsimd)
                    for half in range(2):
                        i = 2 * ip + half
                        ps = psu.tile([P, CH], F32, tag="psu")
                        for k in range(KH):
                            nc.tensor.matmul(
                                ps[:],
                                wupb[:, half, k],
                                ntc[:, k],
                                start=(k == 0),
                                stop=(k == KH - 1),
                            )
                        a = actp.tile([P, CH], F16, tag="act", name=f"act{i}")
                        nc.scalar.activation(
                            a[:],
                            ps[:],
                            mybir.ActivationFunctionType.Gelu,
                            bias=bup_sb[:, i : i + 1],
                            scale=1.0,
                        )
                        acts.append(a)
                        # gelu-gated prefetches: wdn/nt are needed only at
                        # the down phase (~100us later); gating them at late
                        # gelus keeps the early fabric bandwidth for the wup
                        # stream. The wup stream itself is slot-gated via
                        # the NPRE-deep pool (~35us of lead), so it needs no
                        # gelu gating.
                        # On chunk 0 the wdnp/ntp pool slots are empty, so
                        # every un-gated wdn/nt prefetch would hoist to t0;
                        # chunk 0 therefore gates wdn0..3 explicitly. Later
                        # chunks are naturally gated by slot reuse.
                        sched = {12: "wdn0", 18: "wdn1", 24: "wdn2", 30: "nt"}
                        if c == 0:
                            sched[36] = "wdn3"
                        ev = sched.get(i)
                        if ev == "nt":
                            if c + 1 < NCH:
                                fetch_nt(c + 1, gate=a)
                        elif ev == "wdn0":
                            wdn_pre.append(issue_wdn(0, nc.scalar, gate=a))
                        elif ev == "wdn1":
                            wdn_pre.append(issue_wdn(1, nc.scalar, gate=a))
                        elif ev == "wdn2":
                            wdn_pre.append(issue_wdn(2, nc.gpsimd, gate=a))
                        elif ev == "wdn3":
                            wdn_pre.append(issue_wdn(3, nc.gpsimd, gate=a))

                # ---- down projection over chunk c ----
                for m in range(KH):
                    if m < len(wdn_pre):
                        wdnb = wdn_pre[m]
                    else:
                        wdnb = issue_wdn(m, nc.scalar if m % 3 == 0 else nc.gpsimd)
                    ps2 = psd.tile([P, CH], F32, tag="psd")
                    for i in range(KI):
                        nc.tensor.matmul(
                            ps2[:],
                            wdnb[:, i // ISUB, i % ISUB],
                            acts[i][:],
                            start=(i == 0),
                            stop=(i == KI - 1),
                        )
                    osb = outp.tile([P, CH], F32, tag="osb")
                    tok = slice(c * CH, (c + 1) * CH)
                    out_eng = nc.sync
                    if c == NCH - 1 and m == KH - 1:
                        # split the final output so copy/DMA pipeline
                        HC = CH // 2
                        for q in range(2):
                            lo, hi = q * HC, (q + 1) * HC
                            nc.vector.tensor_copy(out=osb[:, lo:hi], in_=ps2[:, lo:hi])
                            out_eng.dma_start(
                                out=ot[m][:, c * CH + lo : c * CH + hi],
                                in_=osb[:, lo:hi],
                            )
                    else:
                        nc.vector.tensor_copy(out=osb[:], in_=ps2[:])
                        out_eng.dma_start(out=ot[m][:, tok], in_=osb[:])
    nc.compile()
    return nc


_CACHED = {}


def _get_nc():
    if "nc" not in _CACHED:
        _CACHED["nc"] = build_bass()
    return _CACHED["nc"]


def _prep_host(x, residual, ln_w, W_up, b_up, W_down):
    """Host-side prep: h, chunk-repacked fp16 normed^T per core, weights."""
    h = x + residual                                   # [B,S,H] f32
    hf = h.reshape(-1, H)                              # [16384, H]
    var = np.mean(np.square(hf), axis=-1)              # f32
    s = 1.0 / np.sqrt(var + EPS)                       # f32
    normed = (hf * s[:, None]).astype(np.float16)      # ln_w folded into W

    Wup_p = (W_up * ln_w[:, None]).astype(np.float16)  # [H, I]
    # wup[ip, p, b, k, il] = Wup_p[k*128+p, (2*ip+b)*128+il]
    WUP = np.ascontiguousarray(
        Wup_p.reshape(KH, P, KI // 2, 2, P).transpose(2, 1, 3, 0, 4)
    )                                                  # [KI/2,P,2,KH,P] f16
    # wdn[m, p, ic, isub, cc] = W_down[(ic*ISUB+isub)*128+p, m*128+cc]
    WDN = np.ascontiguousarray(
        W_down.astype(np.float16).reshape(IC, ISUB, P, KH, P).transpose(3, 2, 0, 1, 4)
    )                                                  # [KH,P,IC,ISUB,P] f16

    # bup[p, i] = b_up[i*128+p] -> contiguous [P, KI] DMA
    BUP = np.ascontiguousarray(b_up.astype(np.float32).reshape(KI, P).T)

    in_maps = []
    for c in range(NCORES):
        blk = normed[c * T_CORE : (c + 1) * T_CORE]    # [T_CORE, H] f16
        # nt[ch, p, k, j] = normed^T[k*128+p, ch*512+j]
        ntc = np.ascontiguousarray(
            blk.T.reshape(KH, P, NCH, CH).transpose(2, 1, 0, 3)
        )                                              # [NCH,P,KH,CH] f16
        in_maps.append({"nt": ntc, "wup": WUP, "wdn": WDN, "bup": BUP})
    return h, in_maps


_RESET_DONE = {}


def _maybe_reset_device():
    """Best-effort terminal NRT reset so a previously wedged device can't
    hang the run. No-op when the axon .so or symbol is unavailable."""
    if _RESET_DONE:
        return
    _RESET_DONE["done"] = True
    try:
        import ctypes
        import jax

        jax.devices()
        lib = ctypes.CDLL("/opt/axon/libaxon_pjrt.so")
        if hasattr(lib, "axon_reset"):
            lib.axon_reset.restype = ctypes.c_int64
            lib.axon_reset()
    except Exception:
        pass


def _run(in_maps, **kw):
    _maybe_reset_device()
    nc = _get_nc()
    return run_bass_kernel_spmd(nc, in_maps, core_ids=list(range(NCORES)), **kw)


def _assemble(results, b_down):
    # ot[m, p, t] -> out[t, m*128+p]
    outs = [r["ot"].transpose(2, 0, 1).reshape(T_CORE, H) for r in results]
    out = np.concatenate(outs, axis=0).reshape(B, S, H)
    out = out + b_down.astype(np.float32)
    return out


def kernel(x, residual, ln_w, W_up, b_up, W_down, b_down):
    x = np.asarray(x, dtype=np.float32)
    residual = np.asarray(residual, dtype=np.float32)
    ln_w = np.asarray(ln_w, dtype=np.float32)
    W_up = np.asarray(W_up, dtype=np.float32)
    b_up = np.asarray(b_up, dtype=np.float32)
    W_down = np.asarray(W_down, dtype=np.float32)
    b_down = np.asarray(b_down, dtype=np.float32)

    h, in_maps = _prep_host(x, residual, ln_w, W_up, b_up, W_down)
    res = _run(in_maps)
    out = _assemble(res.results, b_down)
    return out, h


def kernel_traced(x, residual, ln_w, W_up, b_up, W_down, b_down, **kw):
    """Like kernel() but with NTFF tracing; returns ((out, h), results)."""
    h, in_maps = _prep_host(
        np.asarray(x, np.float32),
        np.asarray(residual, np.float32),
        np.asarray(ln_w, np.float32),
        np.asarray(W_up, np.float32),
        np.asarray(b_up, np.float32),
        np.asarray(W_down, np.float32),
    )
    res = _run(in_maps, trace=True, **kw)
    out = _assemble(res.results, np.asarray(b_down, np.float32))
    return (out, h), res



# revision 21
# speedup vs baseline: 1.1106x; 1.0187x over previous
"""Trainium2 Bass kernel for NeuronLlama4VisionMLP (fused residual-add +
RMSNorm + up-proj + GELU + down-proj).

Distribution: data-parallel over the 16384 tokens -> 2048 tokens per core,
full weights replicated per core, no collectives.

Host side (cheap elementwise / repack prep):
  - h = x + residual  (this is also the module's second output)
  - per-token rsqrt(mean(h^2)+eps) scale and ln_w are folded into the
    device inputs: normed = h * s, W_up' = ln_w[:,None] * W_up
  - normed is shipped transposed + chunk-repacked fp16 so each chunk is
    one fully HBM-contiguous DMA; the device returns out^T per m-tile and
    b_down is added on host.

Device side per core (T=2048 tokens, H=1408 -> KH=11 tiles, I=5632 ->
KI=44 tiles), processed as 4 passes over 512-token chunks; everything in
fp16 so every matmul runs at the 1 col/cycle @2.4GHz peak (f32r weights
pay ~+11ns/MM in LDWEIGHTS):
    up:   psum[i, c] = sum_k wup[k, i].T @ nt[k, c]      (11-MM chains)
    gelu: act[i, c] = Gelu(psum + b_up[i])               (scalar engine)
    down: psum[m, c] = sum_i wdn[i, m].T @ act[i, c]     (44-MM chains)
    out^T[m, c] -> HBM (f32)

Weights are re-streamed per chunk pass (4x wup, 4x wdn ~ 127MB/core
total) -- the ~360GB/s/core HBM fabric absorbs it, so never the
bottleneck. In exchange act SBUF drops to 5.9MB.

Scheduling learned the hard way (see per-queue notes in build_bass):
the three DMA queues (sync/scalar/gpsimd) share ~360GB/s round-robin
at packet granularity, so queue ORDER alone cannot prioritize -- and
the Tile scheduler hoists any dma_start with free pool slots to t0
regardless of python issue order. Every non-critical prefetch is
therefore gated by a real dependency (1-element vector copy into the
prefetch tile -> WAW dep on its DMA): prologue wup pairs gate on bytes
of the critical nt0/w0 pieces, wdn/nt chunk prefetches gate on mid-
chunk gelu outputs. Chain 0 then streams data-paced from ~12us while
NWARM junk matmuls bridge the HAM clock gate (1.2->2.4GHz) until the
real MM stream is dense (~15us); total mm-stream gaps measure ~2us
over the 836us span. Critical prologue pieces are kept >=0.35MB (per-
DMA completion costs ~0.7us/queue) and balanced to the measured queue
shares (gpsimd ~165, scalar ~105, sync ~55-90 GB/s). Outs ride sync;
psum->sbuf copies ride the idle vector engine; scalar runs the gelus.
Note: back-to-back runs trip the P0 power throttle (PE 2.4->2.0GHz,
+20% wall); benchmark on a cool device.
"""
import sys

sys.path.insert(0, "/opt/trn_rl_repo")

import numpy as np
import ml_dtypes
import concourse.bass as bass
from concourse import bacc
import concourse.mybir as mybir
from concourse.tile import TileContext
from concourse.bass_utils import run_bass_kernel_spmd

# Problem shape (hardcoded per contract)
B, S, H, I = 16, 1024, 1408, 5632
EPS = 1e-6
NCORES = 8
P = 128
T_CORE = (B * S) // NCORES       # 2048 tokens per core
KH = H // P                      # 11 k-tiles of H
KI = I // P                      # 44 k-tiles of I
IC = 4                           # i-chunks in down weight repack
ISUB = KI // IC                  # 11 i-subtiles per chunk
CH = 512                         # token chunk width (= max fp32-psum MM N)
NCH = T_CORE // CH               # 4 chunk passes per core
NTSPLIT = 6                      # nt chunk DMA k-split
NWARM = 15                       # HAM warmup matmuls
NPRE = 8                         # wup pair-tiles prefetched in the prologue

F16 = mybir.dt.float16
F32 = mybir.dt.float32




def build_bass():
    nc = bacc.Bacc(None, target_bir_lowering=False)

    # all weight/act DMAs are HBM-contiguous with multi-KB per-partition runs
    nt = nc.declare_dram_parameter("nt", [NCH, P, KH, CH], F16, isOutput=False)
    wup = nc.declare_dram_parameter("wup", [KI // 2, P, 2, KH, P], F16, isOutput=False)
    wdn = nc.declare_dram_parameter("wdn", [KH, P, IC, ISUB, P], F16, isOutput=False)
    # host-transposed [P, KI] so the DMA is one contiguous run per
    # partition; a "(i p) -> p i" rearrange of the flat vector emits 5632
    # single-element descriptors that clog the issuing queue for ~20us
    bup = nc.declare_dram_parameter("bup", [P, KI], F32, isOutput=False)
    ot = nc.declare_dram_parameter("ot", [KH, P, T_CORE], F32, isOutput=True)

    with TileContext(nc) as tc:
        with (
            tc.tile_pool(name="const", bufs=1) as constp,
            tc.tile_pool(name="ntp", bufs=2) as ntp,
            tc.tile_pool(name="wupp", bufs=NPRE) as wupp,
            tc.tile_pool(name="wdnp", bufs=4) as wdnp,
            tc.tile_pool(name="actp", bufs=KI + 2) as actp,
            tc.tile_pool(name="outp", bufs=4) as outp,
            tc.tile_pool(name="psu", bufs=4, space="PSUM") as psu,
            tc.tile_pool(name="psd", bufs=4, space="PSUM") as psd,
        ):
            # Measured per-queue DMA throughput: gpsimd (SWDGE) ~200GB/s;
            # sync/scalar (HWDGE) only ~70-90GB/s each. Transfers issued on
            # one queue share its bandwidth concurrently, and throughput
            # also degrades below ~2KB per-partition runs. So: steady-state
            # weight tiles move as full-tile single descriptors (5.6-11.3KB
            # runs) weighted 2:1 toward gpsimd, and the prologue hand-
            # schedules the first ~3MB by need-time with the critical bytes
            # leading the gpsimd queue. Non-critical prefetches are gated
            # behind the first gelus (the in-order scalar queue can't reach
            # their dma_starts before the preceding ACTIVATE retires).
            nt_tiles = {}

            # The Tile scheduler orders instructions by dependency, not by
            # python issue order: a "gelu-gated" dma_start with free pool
            # slots gets hoisted to t0 and its bytes then compete with the
            # critical prologue stream (seen in traces: wdn0/nt1 dispatch
            # at ~10us). gate_on() forces a real dependency: a 1-element
            # copy from the gate tile into the prefetch tile makes the
            # full-tile DMA WAW-dependent on the gate tile's producer.
            def fetch_nt(c, gate=None):
                t = ntp.tile([P, KH, CH], F16, tag="nt", name=f"nt{c}")
                if gate is not None:
                    nc.vector.tensor_copy(out=t[0:1, 0:1, 0:1], in_=gate[0:1, 0:1])
                nc.scalar.dma_start(out=t[:], in_=nt[c])
                nt_tiles[c] = t

            def issue_wup(ip, eng):
                t = wupp.tile([P, 2, KH, P], F16, tag="wup", name=f"wup{ip}")
                eng.dma_start(out=t[:], in_=wup[ip])
                return t

            def issue_wdn(m, eng, gate=None):
                t = wdnp.tile([P, IC, ISUB, P], F16, tag="wdn", name=f"wdn{m}")
                if gate is not None:
                    nc.vector.tensor_copy(
                        out=t[0:1, 0:1, 0:1, 0:1], in_=gate[0:1, 0:1]
                    )
                eng.dma_start(out=t[:], in_=wdn[m])
                return t

            # ---- prologue: critical-first, staggered so chain 0 streams
            # data-paced from ~12us. Measured early per-queue shares while
            # all three compete: gpsimd ~165, scalar ~105, sync ~55 GB/s
            # (sync is the runt -- it gets only bup + the last k-slice).
            # wup[0] is split per-half so chain 0 (half 0) isn't gated on
            # the full pair; nt0 lands in five staggered slices roughly in
            # k-order so chain 0's k-MMs trickle in behind the junk warmup.
            # The remaining prologue pairs wup[1..NPRE-1] follow in
            # need-order (one pair per 4.75us); in-loop pairs are
            # slot-gated by the NPRE-deep pool (~35us of lead).
            wup_pre = {}
            nt0 = ntp.tile([P, KH, CH], F16, tag="nt", name="nt0")
            nt_tiles[0] = nt0
            wpre = []
            for ip in range(NPRE):
                wpre.append(wupp.tile([P, 2, KH, P], F16, tag="wup", name=f"wup{ip}"))
                wup_pre[ip] = wpre[ip]
            bup_sb = constp.tile([P, KI], F32)
            # critical bytes (nt0 + the wup[0] pair + bup) lead each queue;
            # pieces kept >=0.35MB (smaller slices pay ~0.7us per-DMA
            # completion overhead per queue and tank queue throughput)
            nc.gpsimd.dma_start(out=nt0[:, 0:3], in_=nt[0][:, 0:3])
            nc.scalar.dma_start(out=wpre[0][:, 0], in_=wup[0][:, 0])
            nc.sync.dma_start(out=bup_sb[:], in_=bup[:, 0:KI])
            nc.sync.dma_start(out=nt0[:, 9:KH], in_=nt[0][:, 9:KH])
            nc.gpsimd.dma_start(out=nt0[:, 3:6], in_=nt[0][:, 3:6])
            nc.scalar.dma_start(out=wpre[0][:, 1], in_=wup[0][:, 1])
            nc.scalar.dma_start(out=nt0[:, 6:9], in_=nt[0][:, 6:9])
            # rest of the prologue wup pairs, need-ordered across queues.
            # Each is gated on a byte of a critical piece (1-elem copy ->
            # WAW dep) so the SDMA engines don't round-robin any of these
            # 0.72MB pairs ahead of the critical tail; they dispatch as the
            # criticals complete, still ~5-10us before their chains.
            def gate_pair(ip, gate_ap):
                nc.vector.tensor_copy(out=wpre[ip][0:1, 0:1, 0:1, 0:1], in_=gate_ap)

            g_early = nt0[0:1, 0:1, 0:1]     # lands ~13us
            g_mid = nt0[0:1, 3:4, 0:1]       # lands ~15us
            g_late = nt0[0:1, 6:7, 0:1]      # lands ~17us
            gate_pair(1, g_early)
            nc.gpsimd.dma_start(out=wpre[1][:], in_=wup[1])
            gate_pair(2, g_early)
            nc.scalar.dma_start(out=wpre[2][:], in_=wup[2])
            gate_pair(3, g_mid)
            nc.gpsimd.dma_start(out=wpre[3][:], in_=wup[3])
            gate_pair(4, g_mid)
            nc.sync.dma_start(out=wpre[4][:], in_=wup[4])
            gate_pair(5, g_late)
            nc.gpsimd.dma_start(out=wpre[5][:], in_=wup[5])
            gate_pair(6, g_late)
            nc.scalar.dma_start(out=wpre[6][:], in_=wup[6])
            gate_pair(7, g_late)
            nc.sync.dma_start(out=wpre[7][:], in_=wup[7])

            # PE warmup: ~10 junk matmuls flip the HAM clock gate
            # (1.2->2.4GHz) while the prologue DMAs are still in flight,
            # so the first real chains run at full clock.
            wa = constp.tile([P, CH], F16)
            wb = constp.tile([P, P], F16)
            nc.vector.memset(wa[:], 0.0)
            nc.vector.memset(wb[:], 0.0)
            for _ in range(NWARM):
                pw = psu.tile([P, CH], F32, tag="psu", name="pw")
                nc.tensor.matmul(pw[:], wb[:], wa[:], start=True, stop=True)

            for c in range(NCH):
                ntc = nt_tiles.pop(c)

                # ---- up projection + gelu over chunk c ----
                acts = []
                wdn_pre = []
                for ip in range(KI // 2):
                    if ip in wup_pre:
                        wupb = wup_pre.pop(ip)
                    else:
                        wupb = issue_wup(ip, nc.sync if ip % 4 == 0 else nc.gpsimd)
                    for half in range(2):
                        i = 2 * ip + half
                        ps = psu.tile([P, CH], F32, tag="psu")
                        for k in range(KH):
                            nc.tensor.matmul(
                                ps[:],
                                wupb[:, half, k],
                                ntc[:, k],
                                start=(k == 0),
                                stop=(k == KH - 1),
                            )
                        a = actp.tile([P, CH], F16, tag="act", name=f"act{i}")
                        nc.scalar.activation(
                            a[:],
                            ps[:],
                            mybir.ActivationFunctionType.Gelu,
                            bias=bup_sb[:, i : i + 1],
                            scale=1.0,
                        )
                        acts.append(a)
                        # gelu-gated prefetches: wdn/nt are needed only at
                        # the down phase (~100us later); gating them at late
                        # gelus keeps the early fabric bandwidth for the wup
                        # stream. The wup stream itself is slot-gated via
                        # the NPRE-deep pool (~35us of lead), so it needs no
                        # gelu gating.
                        # On chunk 0 the wdnp/ntp pool slots are empty, so
                        # every un-gated wdn/nt prefetch would hoist to t0;
                        # chunk 0 therefore gates wdn0..3 explicitly. Later
                        # chunks are naturally gated by slot reuse.
                        sched = {12: "wdn0", 18: "wdn1", 24: "wdn2", 30: "nt"}
                        if c == 0:
                            sched[36] = "wdn3"
                        ev = sched.get(i)
                        if ev == "nt":
                            if c + 1 < NCH:
                                fetch_nt(c + 1, gate=a)
                        elif ev == "wdn0":
                            wdn_pre.append(issue_wdn(0, nc.scalar, gate=a))
                        elif ev == "wdn1":
                            wdn_pre.append(issue_wdn(1, nc.scalar, gate=a))
                        elif ev == "wdn2":
                            wdn_pre.append(issue_wdn(2, nc.gpsimd, gate=a))
                        elif ev == "wdn3":
                            wdn_pre.append(issue_wdn(3, nc.gpsimd, gate=a))

                # ---- down projection over chunk c ----
                for m in range(KH):
                    if m < len(wdn_pre):
                        wdnb = wdn_pre[m]
                    else:
                        wdnb = issue_wdn(m, nc.scalar if m % 3 == 0 else nc.gpsimd)
                    tok = slice(c * CH, (c + 1) * CH)
                    out_eng = nc.sync
                    if c == NCH - 1 and m == KH - 1:
                        # column-split the FINAL chain into two 256-wide
                        # psum chains: the first half's copy+DMA+receipt
                        # overlaps the second half-chain's MMs, so only a
                        # 256-col copy+DMA trails the very last matmul.
                        # (Costs 44 extra MM issues at N=256 ~= +0.1us of
                        # PE time; saves ~1.5-2us of tail.)
                        HC = CH // 2
                        for hx in range(2):
                            lo, hi = hx * HC, (hx + 1) * HC
                            ps2h = psd.tile([P, HC], F32, tag="psd")
                            for i in range(KI):
                                nc.tensor.matmul(
                                    ps2h[:],
                                    wdnb[:, i // ISUB, i % ISUB],
                                    acts[i][:, lo:hi],
                                    start=(i == 0),
                                    stop=(i == KI - 1),
                                )
                            osbh = outp.tile([P, HC], F32, tag="osb")
                            nc.vector.tensor_copy(out=osbh[:], in_=ps2h[:])
                            out_eng.dma_start(
                                out=ot[m][:, c * CH + lo : c * CH + hi],
                                in_=osbh[:],
                            )
                    else:
                        ps2 = psd.tile([P, CH], F32, tag="psd")
                        for i in range(KI):
                            nc.tensor.matmul(
                                ps2[:],
                                wdnb[:, i // ISUB, i % ISUB],
                                acts[i][:],
                                start=(i == 0),
                                stop=(i == KI - 1),
                            )
                        osb = outp.tile([P, CH], F32, tag="osb")
                        nc.vector.tensor_copy(out=osb[:], in_=ps2[:])
                        out_eng.dma_start(out=ot[m][:, tok], in_=osb[:])
    nc.compile()
    return nc


_CACHED = {}


def _get_nc():
    if "nc" not in _CACHED:
        _CACHED["nc"] = build_bass()
    return _CACHED["nc"]


def _prep_host(x, residual, ln_w, W_up, b_up, W_down):
    """Host-side prep: h, chunk-repacked fp16 normed^T per core, weights."""
    h = x + residual                                   # [B,S,H] f32
    hf = h.reshape(-1, H)                              # [16384, H]
    var = np.mean(np.square(hf), axis=-1)              # f32
    s = 1.0 / np.sqrt(var + EPS)                       # f32
    normed = (hf * s[:, None]).astype(np.float16)      # ln_w folded into W

    Wup_p = (W_up * ln_w[:, None]).astype(np.float16)  # [H, I]
    # wup[ip, p, b, k, il] = Wup_p[k*128+p, (2*ip+b)*128+il]
    WUP = np.ascontiguousarray(
        Wup_p.reshape(KH, P, KI // 2, 2, P).transpose(2, 1, 3, 0, 4)
    )                                                  # [KI/2,P,2,KH,P] f16
    # wdn[m, p, ic, isub, cc] = W_down[(ic*ISUB+isub)*128+p, m*128+cc]
    WDN = np.ascontiguousarray(
        W_down.astype(np.float16).reshape(IC, ISUB, P, KH, P).transpose(3, 2, 0, 1, 4)
    )                                                  # [KH,P,IC,ISUB,P] f16

    # bup[p, i] = b_up[i*128+p] -> contiguous [P, KI] DMA
    BUP = np.ascontiguousarray(b_up.astype(np.float32).reshape(KI, P).T)

    in_maps = []
    for c in range(NCORES):
        blk = normed[c * T_CORE : (c + 1) * T_CORE]    # [T_CORE, H] f16
        # nt[ch, p, k, j] = normed^T[k*128+p, ch*512+j]
        ntc = np.ascontiguousarray(
            blk.T.reshape(KH, P, NCH, CH).transpose(2, 1, 0, 3)
        )                                              # [NCH,P,KH,CH] f16
        in_maps.append({"nt": ntc, "wup": WUP, "wdn": WDN, "bup": BUP})
    return h, in_maps


_RESET_DONE = {}


def _maybe_reset_device():
    """Best-effort terminal NRT reset so a previously wedged device can't
    hang the run. No-op when the axon .so or symbol is unavailable."""
    if _RESET_DONE:
        return
    _RESET_DONE["done"] = True
    try:
        import ctypes
        import jax

        jax.devices()
        lib = ctypes.CDLL("/opt/axon/libaxon_pjrt.so")
        if hasattr(lib, "axon_reset"):
            lib.axon_reset.restype = ctypes.c_int64
            lib.axon_reset()
    except Exception:
        pass


def _run(in_maps, **kw):
    _maybe_reset_device()
    nc = _get_nc()
    return run_bass_kernel_spmd(nc, in_maps, core_ids=list(range(NCORES)), **kw)


def _assemble(results, b_down):
    # ot[m, p, t] -> out[t, m*128+p]
    outs = [r["ot"].transpose(2, 0, 1).reshape(T_CORE, H) for r in results]
    out = np.concatenate(outs, axis=0).reshape(B, S, H)
    out = out + b_down.astype(np.float32)
    return out


def kernel(x, residual, ln_w, W_up, b_up, W_down, b_down):
    x = np.asarray(x, dtype=np.float32)
    residual = np.asarray(residual, dtype=np.float32)
    ln_w = np.asarray(ln_w, dtype=np.float32)
    W_up = np.asarray(W_up, dtype=np.float32)
    b_up = np.asarray(b_up, dtype=np.float32)
    W_down = np.asarray(W_down, dtype=np.float32)
    b_down = np.asarray(b_down, dtype=np.float32)

    h, in_maps = _prep_host(x, residual, ln_w, W_up, b_up, W_down)
    res = _run(in_maps)
    out = _assemble(res.results, b_down)
    return out, h


def kernel_traced(x, residual, ln_w, W_up, b_up, W_down, b_down, **kw):
    """Like kernel() but with NTFF tracing; returns ((out, h), results)."""
    h, in_maps = _prep_host(
        np.asarray(x, np.float32),
        np.asarray(residual, np.float32),
        np.asarray(ln_w, np.float32),
        np.asarray(W_up, np.float32),
        np.asarray(b_up, np.float32),
        np.asarray(W_down, np.float32),
    )
    res = _run(in_maps, trace=True, **kw)
    out = _assemble(res.results, np.asarray(b_down, np.float32))
    return (out, h), res



# revision 23
# speedup vs baseline: 1.1108x; 1.0002x over previous
"""Trainium2 Bass kernel for NeuronLlama4VisionMLP (fused residual-add +
RMSNorm + up-proj + GELU + down-proj).

Distribution: data-parallel over the 16384 tokens -> 2048 tokens per core,
full weights replicated per core, no collectives.

Host side (cheap elementwise / repack prep):
  - h = x + residual  (this is also the module's second output)
  - per-token rsqrt(mean(h^2)+eps) scale and ln_w are folded into the
    device inputs: normed = h * s, W_up' = ln_w[:,None] * W_up
  - normed is shipped transposed + chunk-repacked fp16 so each chunk is
    one fully HBM-contiguous DMA; the device returns out^T per m-tile and
    b_down is added on host.

Device side per core (T=2048 tokens, H=1408 -> KH=11 tiles, I=5632 ->
KI=44 tiles), processed as 4 passes over 512-token chunks; everything in
fp16 so every matmul runs at the 1 col/cycle @2.4GHz peak (f32r weights
pay ~+11ns/MM in LDWEIGHTS):
    up:   psum[i, c] = sum_k wup[k, i].T @ nt[k, c]      (11-MM chains)
    gelu: act[i, c] = Gelu(psum + b_up[i])               (scalar engine)
    down: psum[m, c] = sum_i wdn[i, m].T @ act[i, c]     (44-MM chains)
    out^T[m, c] -> HBM (f32)

Weights are re-streamed per chunk pass (4x wup, 4x wdn ~ 127MB/core
total) -- the ~360GB/s/core HBM fabric absorbs it, so never the
bottleneck. In exchange act SBUF drops to 5.9MB.

Scheduling learned the hard way (see per-queue notes in build_bass):
the three DMA queues (sync/scalar/gpsimd) share ~360GB/s round-robin
at packet granularity, so queue ORDER alone cannot prioritize -- and
the Tile scheduler hoists any dma_start with free pool slots to t0
regardless of python issue order. Every non-critical prefetch is
therefore gated by a real dependency (1-element vector copy into the
prefetch tile -> WAW dep on its DMA): prologue wup pairs gate on bytes
of the critical nt0/w0 pieces, wdn/nt chunk prefetches gate on mid-
chunk gelu outputs. Chain 0 then streams data-paced from ~12us while
NWARM junk matmuls bridge the HAM clock gate (1.2->2.4GHz) until the
real MM stream is dense (~15us); total mm-stream gaps measure ~2us
over the 836us span. Critical prologue pieces are kept >=0.35MB (per-
DMA completion costs ~0.7us/queue) and balanced to the measured queue
shares (gpsimd ~165, scalar ~105, sync ~55-90 GB/s). Outs ride sync;
psum->sbuf copies ride the idle vector engine; scalar runs the gelus.
Note: back-to-back runs trip the P0 power throttle (PE 2.4->2.0GHz,
+20% wall); benchmark on a cool device.
"""
import sys

sys.path.insert(0, "/opt/trn_rl_repo")

import numpy as np
import ml_dtypes
import concourse.bass as bass
from concourse import bacc
import concourse.mybir as mybir
from concourse.tile import TileContext
from concourse.bass_utils import run_bass_kernel_spmd

# Problem shape (hardcoded per contract)
B, S, H, I = 16, 1024, 1408, 5632
EPS = 1e-6
NCORES = 8
P = 128
T_CORE = (B * S) // NCORES       # 2048 tokens per core
KH = H // P                      # 11 k-tiles of H
KI = I // P                      # 44 k-tiles of I
IC = 4                           # i-chunks in down weight repack
ISUB = KI // IC                  # 11 i-subtiles per chunk
CH = 512                         # token chunk width (= max fp32-psum MM N)
NCH = T_CORE // CH               # 4 chunk passes per core
NTSPLIT = 6                      # nt chunk DMA k-split
NWARM = 15                       # HAM warmup matmuls
NPRE = 8                         # wup pair-tiles prefetched in the prologue

F16 = mybir.dt.float16
F32 = mybir.dt.float32




def build_bass():
    nc = bacc.Bacc(None, target_bir_lowering=False)

    # all weight/act DMAs are HBM-contiguous with multi-KB per-partition runs
    nt = nc.declare_dram_parameter("nt", [NCH, P, KH, CH], F16, isOutput=False)
    wup = nc.declare_dram_parameter("wup", [KI // 2, P, 2, KH, P], F16, isOutput=False)
    wdn = nc.declare_dram_parameter("wdn", [KH, P, IC, ISUB, P], F16, isOutput=False)
    # host-transposed [P, KI] so the DMA is one contiguous run per
    # partition; a "(i p) -> p i" rearrange of the flat vector emits 5632
    # single-element descriptors that clog the issuing queue for ~20us
    bup = nc.declare_dram_parameter("bup", [P, KI], F32, isOutput=False)
    ot = nc.declare_dram_parameter("ot", [KH, P, T_CORE], F32, isOutput=True)

    with TileContext(nc) as tc:
        with (
            tc.tile_pool(name="const", bufs=1) as constp,
            tc.tile_pool(name="ntp", bufs=2) as ntp,
            tc.tile_pool(name="wupp", bufs=NPRE) as wupp,
            tc.tile_pool(name="wdnp", bufs=4) as wdnp,
            tc.tile_pool(name="actp", bufs=KI + 2) as actp,
            tc.tile_pool(name="outp", bufs=4) as outp,
            tc.tile_pool(name="psu", bufs=4, space="PSUM") as psu,
            tc.tile_pool(name="psd", bufs=4, space="PSUM") as psd,
        ):
            # Measured per-queue DMA throughput: gpsimd (SWDGE) ~200GB/s;
            # sync/scalar (HWDGE) only ~70-90GB/s each. Transfers issued on
            # one queue share its bandwidth concurrently, and throughput
            # also degrades below ~2KB per-partition runs. So: steady-state
            # weight tiles move as full-tile single descriptors (5.6-11.3KB
            # runs) weighted 2:1 toward gpsimd, and the prologue hand-
            # schedules the first ~3MB by need-time with the critical bytes
            # leading the gpsimd queue. Non-critical prefetches are gated
            # behind the first gelus (the in-order scalar queue can't reach
            # their dma_starts before the preceding ACTIVATE retires).
            nt_tiles = {}

            # The Tile scheduler orders instructions by dependency, not by
            # python issue order: a "gelu-gated" dma_start with free pool
            # slots gets hoisted to t0 and its bytes then compete with the
            # critical prologue stream (seen in traces: wdn0/nt1 dispatch
            # at ~10us). gate_on() forces a real dependency: a 1-element
            # copy from the gate tile into the prefetch tile makes the
            # full-tile DMA WAW-dependent on the gate tile's producer.
            def fetch_nt(c, gate=None):
                t = ntp.tile([P, KH, CH], F16, tag="nt", name=f"nt{c}")
                if gate is not None:
                    nc.vector.tensor_copy(out=t[0:1, 0:1, 0:1], in_=gate[0:1, 0:1])
                nc.scalar.dma_start(out=t[:], in_=nt[c])
                nt_tiles[c] = t

            def issue_wup(ip, eng):
                t = wupp.tile([P, 2, KH, P], F16, tag="wup", name=f"wup{ip}")
                eng.dma_start(out=t[:], in_=wup[ip])
                return t

            def issue_wdn(m, eng, gate=None):
                t = wdnp.tile([P, IC, ISUB, P], F16, tag="wdn", name=f"wdn{m}")
                if gate is not None:
                    nc.vector.tensor_copy(
                        out=t[0:1, 0:1, 0:1, 0:1], in_=gate[0:1, 0:1]
                    )
                eng.dma_start(out=t[:], in_=wdn[m])
                return t

            # ---- prologue: critical-first, staggered so chain 0 streams
            # data-paced from ~12us. Measured early per-queue shares while
            # all three compete: gpsimd ~165, scalar ~105, sync ~55 GB/s
            # (sync is the runt -- it gets only bup + the last k-slice).
            # wup[0] is split per-half so chain 0 (half 0) isn't gated on
            # the full pair; nt0 lands in five staggered slices roughly in
            # k-order so chain 0's k-MMs trickle in behind the junk warmup.
            # The remaining prologue pairs wup[1..NPRE-1] follow in
            # need-order (one pair per 4.75us); in-loop pairs are
            # slot-gated by the NPRE-deep pool (~35us of lead).
            wup_pre = {}
            nt0 = ntp.tile([P, KH, CH], F16, tag="nt", name="nt0")
            nt_tiles[0] = nt0
            wpre = []
            for ip in range(NPRE):
                wpre.append(wupp.tile([P, 2, KH, P], F16, tag="wup", name=f"wup{ip}"))
                wup_pre[ip] = wpre[ip]
            bup_sb = constp.tile([P, KI], F32)
            # critical bytes (nt0 + the wup[0] pair + bup) lead each queue;
            # pieces kept >=0.35MB (smaller slices pay ~0.7us per-DMA
            # completion overhead per queue and tank queue throughput)
            nc.gpsimd.dma_start(out=nt0[:, 0:6], in_=nt[0][:, 0:6])
            nc.scalar.dma_start(out=wpre[0][:, 0], in_=wup[0][:, 0])
            nc.sync.dma_start(out=bup_sb[:], in_=bup[:, 0:KI])
            nc.sync.dma_start(out=nt0[:, 9:KH], in_=nt[0][:, 9:KH])
            nc.scalar.dma_start(out=wpre[0][:, 1], in_=wup[0][:, 1])
            nc.scalar.dma_start(out=nt0[:, 6:9], in_=nt[0][:, 6:9])
            # rest of the prologue wup pairs, need-ordered across queues.
            # Each is gated on a byte of a critical piece (1-elem copy ->
            # WAW dep) so the SDMA engines don't round-robin any of these
            # 0.72MB pairs ahead of the critical tail; they dispatch as the
            # criticals complete, still ~5-10us before their chains.
            def gate_pair(ip, gate_ap):
                nc.vector.tensor_copy(out=wpre[ip][0:1, 0:1, 0:1, 0:1], in_=gate_ap)

            g_early = nt0[0:1, 0:1, 0:1]     # lands ~14us
            g_mid = nt0[0:1, 3:4, 0:1]       # lands ~14us (same DMA)
            g_late = nt0[0:1, 6:7, 0:1]      # lands ~17us
            # w1 split per-half across the two queues that free up first
            # (sync after its tiny criticals, gpsimd after nt0[0:6]) so
            # chains 2/3 get their halves by ~17-19us; each half gets its
            # own gate byte (the pair-gate byte only covers half 0).
            nc.vector.tensor_copy(out=wpre[1][0:1, 0:1, 0:1, 0:1], in_=g_early)
            nc.sync.dma_start(out=wpre[1][:, 0], in_=wup[1][:, 0])
            nc.vector.tensor_copy(out=wpre[1][0:1, 1:2, 0:1, 0:1], in_=g_early)
            nc.gpsimd.dma_start(out=wpre[1][:, 1], in_=wup[1][:, 1])
            gate_pair(2, g_early)
            nc.scalar.dma_start(out=wpre[2][:], in_=wup[2])
            gate_pair(3, g_mid)
            nc.gpsimd.dma_start(out=wpre[3][:], in_=wup[3])
            gate_pair(4, g_mid)
            nc.sync.dma_start(out=wpre[4][:], in_=wup[4])
            gate_pair(5, g_late)
            nc.gpsimd.dma_start(out=wpre[5][:], in_=wup[5])
            gate_pair(6, g_late)
            nc.scalar.dma_start(out=wpre[6][:], in_=wup[6])
            gate_pair(7, g_late)
            nc.sync.dma_start(out=wpre[7][:], in_=wup[7])

            # PE warmup: ~10 junk matmuls flip the HAM clock gate
            # (1.2->2.4GHz) while the prologue DMAs are still in flight,
            # so the first real chains run at full clock.
            wa = constp.tile([P, CH], F16)
            wb = constp.tile([P, P], F16)
            nc.vector.memset(wa[:], 0.0)
            nc.vector.memset(wb[:], 0.0)
            for _ in range(NWARM):
                pw = psu.tile([P, CH], F32, tag="psu", name="pw")
                nc.tensor.matmul(pw[:], wb[:], wa[:], start=True, stop=True)

            for c in range(NCH):
                ntc = nt_tiles.pop(c)

                # ---- up projection + gelu over chunk c ----
                acts = []
                wdn_pre = []
                for ip in range(KI // 2):
                    if ip in wup_pre:
                        wupb = wup_pre.pop(ip)
                    else:
                        wupb = issue_wup(ip, nc.sync if ip % 4 == 0 else nc.gpsimd)
                    for half in range(2):
                        i = 2 * ip + half
                        ps = psu.tile([P, CH], F32, tag="psu")
                        for k in range(KH):
                            nc.tensor.matmul(
                                ps[:],
                                wupb[:, half, k],
                                ntc[:, k],
                                start=(k == 0),
                                stop=(k == KH - 1),
                            )
                        a = actp.tile([P, CH], F16, tag="act", name=f"act{i}")
                        nc.scalar.activation(
                            a[:],
                            ps[:],
                            mybir.ActivationFunctionType.Gelu,
                            bias=bup_sb[:, i : i + 1],
                            scale=1.0,
                        )
                        acts.append(a)
                        # gelu-gated prefetches: wdn/nt are needed only at
                        # the down phase (~100us later); gating them at late
                        # gelus keeps the early fabric bandwidth for the wup
                        # stream. The wup stream itself is slot-gated via
                        # the NPRE-deep pool (~35us of lead), so it needs no
                        # gelu gating.
                        # On chunk 0 the wdnp/ntp pool slots are empty, so
                        # every un-gated wdn/nt prefetch would hoist to t0;
                        # chunk 0 therefore gates wdn0..3 explicitly. Later
                        # chunks are naturally gated by slot reuse.
                        sched = {12: "wdn0", 18: "wdn1", 24: "wdn2", 30: "nt"}
                        if c == 0:
                            sched[36] = "wdn3"
                        ev = sched.get(i)
                        if ev == "nt":
                            if c + 1 < NCH:
                                fetch_nt(c + 1, gate=a)
                        elif ev == "wdn0":
                            wdn_pre.append(issue_wdn(0, nc.scalar, gate=a))
                        elif ev == "wdn1":
                            wdn_pre.append(issue_wdn(1, nc.scalar, gate=a))
                        elif ev == "wdn2":
                            wdn_pre.append(issue_wdn(2, nc.gpsimd, gate=a))
                        elif ev == "wdn3":
                            wdn_pre.append(issue_wdn(3, nc.gpsimd, gate=a))

                # ---- down projection over chunk c ----
                for m in range(KH):
                    if m < len(wdn_pre):
                        wdnb = wdn_pre[m]
                    else:
                        wdnb = issue_wdn(m, nc.scalar if m % 3 == 0 else nc.gpsimd)
                    tok = slice(c * CH, (c + 1) * CH)
                    out_eng = nc.sync
                    if c == NCH - 1 and m == KH - 1:
                        # column-split the FINAL chain into two 256-wide
                        # psum chains: the first half's copy+DMA+receipt
                        # overlaps the second half-chain's MMs, so only a
                        # 256-col copy+DMA trails the very last matmul.
                        # (Costs 44 extra MM issues at N=256 ~= +0.1us of
                        # PE time; saves ~1.5-2us of tail.)
                        HC = CH // 2
                        for hx in range(2):
                            lo, hi = hx * HC, (hx + 1) * HC
                            ps2h = psd.tile([P, HC], F32, tag="psd")
                            for i in range(KI):
                                nc.tensor.matmul(
                                    ps2h[:],
                                    wdnb[:, i // ISUB, i % ISUB],
                                    acts[i][:, lo:hi],
                                    start=(i == 0),
                                    stop=(i == KI - 1),
                                )
                            osbh = outp.tile([P, HC], F32, tag="osb")
                            nc.vector.tensor_copy(out=osbh[:], in_=ps2h[:])
                            out_eng.dma_start(
                                out=ot[m][:, c * CH + lo : c * CH + hi],
                                in_=osbh[:],
                            )
                    else:
                        ps2 = psd.tile([P, CH], F32, tag="psd")
                        for i in range(KI):
                            nc.tensor.matmul(
                                ps2[:],
                                wdnb[:, i // ISUB, i % ISUB],
                                acts[i][:],
                                start=(i == 0),
                                stop=(i == KI - 1),
                            )
                        osb = outp.tile([P, CH], F32, tag="osb")
                        nc.vector.tensor_copy(out=osb[:], in_=ps2[:])
                        out_eng.dma_start(out=ot[m][:, tok], in_=osb[:])
    nc.compile()
    return nc


_CACHED = {}


def _get_nc():
    if "nc" not in _CACHED:
        _CACHED["nc"] = build_bass()
    return _CACHED["nc"]


def _prep_host(x, residual, ln_w, W_up, b_up, W_down):
    """Host-side prep: h, chunk-repacked fp16 normed^T per core, weights."""
    h = x + residual                                   # [B,S,H] f32
    hf = h.reshape(-1, H)                              # [16384, H]
    var = np.mean(np.square(hf), axis=-1)              # f32
    s = 1.0 / np.sqrt(var + EPS)                       # f32
    normed = (hf * s[:, None]).astype(np.float16)      # ln_w folded into W

    Wup_p = (W_up * ln_w[:, None]).astype(np.float16)  # [H, I]
    # wup[ip, p, b, k, il] = Wup_p[k*128+p, (2*ip+b)*128+il]
    WUP = np.ascontiguousarray(
        Wup_p.reshape(KH, P, KI // 2, 2, P).transpose(2, 1, 3, 0, 4)
    )                                                  # [KI/2,P,2,KH,P] f16
    # wdn[m, p, ic, isub, cc] = W_down[(ic*ISUB+isub)*128+p, m*128+cc]
    WDN = np.ascontiguousarray(
        W_down.astype(np.float16).reshape(IC, ISUB, P, KH, P).transpose(3, 2, 0, 1, 4)
    )                                                  # [KH,P,IC,ISUB,P] f16

    # bup[p, i] = b_up[i*128+p] -> contiguous [P, KI] DMA
    BUP = np.ascontiguousarray(b_up.astype(np.float32).reshape(KI, P).T)

    in_maps = []
    for c in range(NCORES):
        blk = normed[c * T_CORE : (c + 1) * T_CORE]    # [T_CORE, H] f16
        # nt[ch, p, k, j] = normed^T[k*128+p, ch*512+j]
        ntc = np.ascontiguousarray(
            blk.T.reshape(KH, P, NCH, CH).transpose(2, 1, 0, 3)
        )                                              # [NCH,P,KH,CH] f16
        in_maps.append({"nt": ntc, "wup": WUP, "wdn": WDN, "bup": BUP})
    return h, in_maps


_RESET_DONE = {}


def _maybe_reset_device():
    """Best-effort terminal NRT reset so a previously wedged device can't
    hang the run. No-op when the axon .so or symbol is unavailable."""
    if _RESET_DONE:
        return
    _RESET_DONE["done"] = True
    try:
        import ctypes
        import jax

        jax.devices()
        lib = ctypes.CDLL("/opt/axon/libaxon_pjrt.so")
        if hasattr(lib, "axon_reset"):
            lib.axon_reset.restype = ctypes.c_int64
            lib.axon_reset()
    except Exception:
        pass


def _run(in_maps, **kw):
    _maybe_reset_device()
    nc = _get_nc()
    return run_bass_kernel_spmd(nc, in_maps, core_ids=list(range(NCORES)), **kw)


def _assemble(results, b_down):
    # ot[m, p, t] -> out[t, m*128+p]
    outs = [r["ot"].transpose(2, 0, 1).reshape(T_CORE, H) for r in results]
    out = np.concatenate(outs, axis=0).reshape(B, S, H)
    out = out + b_down.astype(np.float32)
    return out


def kernel(x, residual, ln_w, W_up, b_up, W_down, b_down):
    x = np.asarray(x, dtype=np.float32)
    residual = np.asarray(residual, dtype=np.float32)
    ln_w = np.asarray(ln_w, dtype=np.float32)
    W_up = np.asarray(W_up, dtype=np.float32)
    b_up = np.asarray(b_up, dtype=np.float32)
    W_down = np.asarray(W_down, dtype=np.float32)
    b_down = np.asarray(b_down, dtype=np.float32)

    h, in_maps = _prep_host(x, residual, ln_w, W_up, b_up, W_down)
    res = _run(in_maps)
    out = _assemble(res.results, b_down)
    return out, h


def kernel_traced(x, residual, ln_w, W_up, b_up, W_down, b_down, **kw):
    """Like kernel() but with NTFF tracing; returns ((out, h), results)."""
    h, in_maps = _prep_host(
        np.asarray(x, np.float32),
        np.asarray(residual, np.float32),
        np.asarray(ln_w, np.float32),
        np.asarray(W_up, np.float32),
        np.asarray(b_up, np.float32),
        np.asarray(W_down, np.float32),
    )
    res = _run(in_maps, trace=True, **kw)
    out = _assemble(res.results, np.asarray(b_down, np.float32))
    return (out, h), res

